# revision 2
# baseline (speedup 1.0000x reference)
"""Contrastive pair loss on 8 Trainium2 NeuronCores.

loss = mean_b( relu(mean_i((z1[b,i]-z2[b,i])^2) - margin) )  for
z1, z2 of shape (1024, 256, 16, 16) fp32.

Sharding: data-parallel over the batch axis — each of the 8 cores gets 128
rows (one row = 65536 contiguous fp32). On-chip, each core streams the two
32 MiB shards through SBUF in [128, 4096] tiles: DVE computes z1-z2, ACT
computes Square with a per-partition accumulation (accum_out), one slot
per tile; a final DVE reduce collapses the 16 slots to a per-row
sum-of-squares, which is DMA'd out as a [128, 1] tensor. The tiny
hinge/mean epilogue over 1024 row values runs on host.
"""

import numpy as np

B = 1024
CODE = 256 * 16 * 16  # 65536
N_CORES = 8
ROWS = B // N_CORES  # 128 rows per core == SBUF partition count
F = 4096             # columns per tile
NT = CODE // F       # 16 tiles per shard
MARGIN = 0.01

_CACHE = {}


def _split_multi_waits(nc):
    """The walrus build in this image rejects instructions carrying more
    than one sync-wait command ("Too many sync wait commands",
    setupSyncWait). Tile routinely emits several waits on one instruction,
    so split them: for each instruction with N>1 waits, inject N-1
    single-wait NoOps on the same engine immediately before it. Same-engine
    program order makes this semantically identical."""
    from concourse import mybir

    k = 0
    for fn in nc.m.functions:
        for blk in fn.blocks:
            insts = blk.instructions
            out = []
            changed = False
            for ins in insts:
                si = ins.sync_info
                if si is not None and si.on_wait and len(si.on_wait) > 1:
                    waits = list(si.on_wait)
                    for w in waits[:-1]:
                        k += 1
                        nop = mybir.InstNoOp(
                            name=f"WSPLIT-{k}",
                            text_hint="split_wait",
                            bass_nofuse=True,
                        )
                        nop.engine = ins.engine
                        nop.sync_info = mybir.SyncInfo(on_wait=[w], on_update=[])
                        out.append(nop)
                    si.on_wait = waits[-1:]
                    ins.sync_info = si
                    changed = True
                out.append(ins)
            if changed:
                blk.instructions = out


def _build():
    if "nc" in _CACHE:
        return _CACHE["nc"]

    import concourse.bass as bass
    from concourse import mybir
    from concourse.tile import TileContext

    nc = bass.Bass("TRN2", target_bir_lowering=False, num_devices=N_CORES)
    z1 = nc.dram_tensor("z1", [ROWS, CODE], mybir.dt.float32, kind="ExternalInput")
    z2 = nc.dram_tensor("z2", [ROWS, CODE], mybir.dt.float32, kind="ExternalInput")
    out = nc.dram_tensor("out", [ROWS, 1], mybir.dt.float32, kind="ExternalOutput")

    with TileContext(nc) as tc:
        with (
            tc.tile_pool(name="z1p", bufs=3) as p1,
            tc.tile_pool(name="z2p", bufs=3) as p2,
            tc.tile_pool(name="dp", bufs=3) as pd,
            tc.tile_pool(name="st", bufs=1) as ps,
        ):
            acc = ps.tile([ROWS, NT], mybir.dt.float32)
            for j in range(NT):
                t1 = p1.tile([ROWS, F], mybir.dt.float32)
                nc.sync.dma_start(out=t1[:], in_=z1[:, bass.ts(j, F)])
                t2 = p2.tile([ROWS, F], mybir.dt.float32)
                nc.sync.dma_start(out=t2[:], in_=z2[:, bass.ts(j, F)])
                d = pd.tile([ROWS, F], mybir.dt.float32)
                nc.vector.tensor_sub(out=d[:], in0=t1[:], in1=t2[:])
                nc.scalar.activation(
                    out=d[:],
                    in_=d[:],
                    func=mybir.ActivationFunctionType.Square,
                    accum_out=acc[:, j : j + 1],
                )
            rowsum = ps.tile([ROWS, 1], mybir.dt.float32)
            nc.vector.tensor_reduce(
                out=rowsum[:],
                in_=acc[:],
                axis=mybir.AxisListType.X,
                op=mybir.AluOpType.add,
            )
            nc.sync.dma_start(out=out[:], in_=rowsum[:])

    _split_multi_waits(nc)

    _CACHE["nc"] = nc
    return nc


def _run(z1, z2, trace=False):
    from concourse.bass_utils import run_bass_kernel_spmd

    nc = _build()
    z1f = np.ascontiguousarray(np.asarray(z1, dtype=np.float32)).reshape(B, CODE)
    z2f = np.ascontiguousarray(np.asarray(z2, dtype=np.float32)).reshape(B, CODE)
    in_maps = [
        {
            "z1": z1f[c * ROWS : (c + 1) * ROWS],
            "z2": z2f[c * ROWS : (c + 1) * ROWS],
        }
        for c in range(N_CORES)
    ]
    res = run_bass_kernel_spmd(
        nc, in_maps, core_ids=list(range(N_CORES)), trace=trace
    )
    rowsum = np.concatenate(
        [res.results[c]["out"][:, 0] for c in range(N_CORES)]
    ).astype(np.float64)
    hamm = rowsum / CODE
    hinged = np.where(hamm > MARGIN, hamm - MARGIN, 0.0)
    loss = np.float32(hinged.sum() / B)
    return np.asarray(loss, dtype=np.float32), res


def kernel(z1, z2):
    return _run(z1, z2, trace=False)[0]
